# revision 14
# baseline (speedup 1.0000x reference)
"""Windowed sparse attention (16x16 windows, keys from x+skip) on 8 TRN2 NeuronCores.

Reference computation (all 1x1 convs + per-window attention):
  q = Wq @ x;  k,v = split(Wkv @ [x;skip]);  per 16x16 window w/ 256 queries and
  512 keys (256 from x, 256 from skip):  out = softmax(q k^T / 8) v;  y = Wo @ out + bo.

Sharding: each core takes one 16-row strip of the 128x128 image (one window-row X),
both batch elements — all 128 of its windows are fully local; only weights replicated.

Per-core dataflow (fp32r matmuls at full PE rate, transposed-softmax layout):
  - projections produce qT/kT [d, pixels] and v in [pixel, (h d)] layout directly
  - simT[j,i] = kT^T @ qT per window; exp on ScalarE (scale=1/8 folded in)
  - attn@v via lhsT = [v | ones]: softmax denominator s arrives free as psum row 64
  - recip(s) on DVE per window-pair, broadcast via DRAM-bounce DMA,
    normalize-mult on GPSIMD
  - Wo projection from head-stacked normalized outputs, bias fused in ScalarE evac
"""
import sys

if '/opt/trn_rl_repo' not in sys.path:
    sys.path.insert(0, '/opt/trn_rl_repo')

import numpy as np
import concourse.bass as bass
import concourse.tile as tile
import concourse.mybir as mybir
from concourse.bass_utils import run_bass_kernel_spmd

F32 = mybir.dt.float32
F32R = mybir.dt.float32r
AFT = mybir.ActivationFunctionType

N_CORES = 8
B = 2            # batch
C = 256          # model channels
H = 8            # heads
D = 64           # head dim
INNER = H * D    # 512
WIN = 16         # window side
RS = 16          # strip rows per core (= one window row)
WCOL = 128       # image width
PX = RS * WCOL   # 2048 pixels per (batch, strip)
NY = 8           # windows along width
NI = WIN * WIN   # 256 queries per window


def _split_multiwaits(nc, max_waits=1):
    """walrus codegen rejects instructions carrying >1 sem wait (seen on the
    TileContext exit drain); hoist extras onto single-wait NoOps just before."""
    for f in nc.m.functions:
        for blk in f.blocks:
            out, changed = [], False
            for ins in blk.instructions:
                si = ins.sync_info
                if si is not None and len(si.on_wait) > max_waits:
                    waits = list(si.on_wait)
                    SyncInfo = type(si)
                    extra, keep = waits[:-max_waits], waits[-max_waits:]
                    for i, w in enumerate(extra):
                        n = mybir.InstNoOp(name=f"{ins.name}-sw{i}", ins=[], outs=[])
                        n.engine = ins.engine
                        n.sync_info = SyncInfo(on_wait=[w], on_update=[])
                        out.append(n)
                    si.on_wait = keep
                    changed = True
                out.append(ins)
            if changed:
                blk.instructions = out
    return nc


def _bcast_ap(dram_row_ap, parts, free):
    """[free] DRAM row -> [parts, free] replicated read AP (step-0 leading dim)."""
    APcls = type(dram_row_ap)
    return APcls(tensor=dram_row_ap.tensor, offset=dram_row_ap.offset,
                 ap=[[0, parts], [1, free]])


class Ctx:
    pass


def _emit_projections(g, b, y):
    """Stage window pixels and project q/k/v for window-group (b, y)."""
    nc = g.nc
    xw = g.xwpool.tile([128, 2, 2, NI], F32R, name=f"xw{b}_{y}", tag="xw")
    for pi in range(2):
        nc.sync.dma_start(
            xw[:, pi],
            g.parts[pi].rearrange("p kc (r y c) -> p kc y r c",
                                  y=NY, c=WIN)[:, :, y])

    # q: qT [128(2h), NI] x 4 head-pair chunks
    q_y = g.qkpool.tile([128, 4, NI], F32R, name=f"q{b}_{y}", tag="q")
    for mc2 in range(2):
        pq = g.pproj.tile([128, 2, NI], F32, tag="pp")
        for mi in range(2):
            mc = mc2 * 2 + mi
            for kc in range(2):
                nc.tensor.matmul(
                    pq[:, mi], g.wq_sb[:, kc, mc * 128:(mc + 1) * 128],
                    xw[:, 0, kc, :], start=(kc == 0), stop=(kc == 1))
        nc.vector.tensor_copy(q_y[:, mc2 * 2:(mc2 + 1) * 2], pq[:])

    # k: kT [128(2h), 2part, NI] x 4 chunks
    k_y = g.qkpool.tile([128, 4, 2, NI], F32R, name=f"k{b}_{y}", tag="k")
    for kc4 in range(4):
        pk = g.pproj.tile([128, 2, NI], F32, tag="pp")
        for pi in range(2):
            for kc in range(2):
                nc.tensor.matmul(
                    pk[:, pi], g.wk_sb[:, kc, kc4 * 128:(kc4 + 1) * 128],
                    xw[:, pi, kc, :], start=(kc == 0), stop=(kc == 1))
        nc.vector.tensor_copy(k_y[:, kc4], pk[:])

    # v (transposed): [128 j, 8h x (64|ones)] x 4 j-chunks
    v_y = []
    for pi in range(2):
        for jc in range(2):
            pv = g.pproj.tile([128, INNER], F32, tag="pp")
            for kc in range(2):
                nc.tensor.matmul(
                    pv[:], xw[:, pi, kc, jc * 128:(jc + 1) * 128],
                    g.wv_sb[:, kc, :], start=(kc == 0), stop=(kc == 1))
            vt = g.vpool.tile([128, H, D + 1], F32R,
                              name=f"v{b}_{y}_{pi}{jc}", tag="v")
            nc.vector.tensor_copy(vt[:, :, D], nc.const_aps.tensor(1.0, (128, H), F32))
            nc.vector.tensor_copy(
                vt[:, :, 0:D], pv[:].rearrange("p (h d) -> p h d", h=H))
            v_y.append(vt)
    return q_y, k_y, v_y


def _emit_attention(g, b, y, q_y, k_y, v_y):
    """simT -> exp -> (attn@[v|1]) for all 8 heads; returns unnorm [65, H, NI]."""
    nc = g.nc
    un = g.unpool.tile([65, H, NI], F32, name=f"u{b}_{y}", tag="un")
    for hp in range(4):
        # per-head sim tiles (2 banks each) so exp(h) overlaps sim MMs(h+1);
        # even/odd heads packed in the PE array via tile_position row groups
        sims = [g.psim.tile([128, 2, 2, NI], F32, tag="sim", name=f"sim{b}_{y}_{hp}_{h2}")
                for h2 in range(2)]
        for pi in range(2):
            for jh in range(2):
                for h2 in range(2):
                    nc.tensor.matmul(
                        sims[h2][:, pi, jh],
                        k_y[h2 * 64:(h2 + 1) * 64, hp, pi, jh * 128:(jh + 1) * 128],
                        q_y[h2 * 64:(h2 + 1) * 64, hp],
                        start=True, stop=True, tile_position=(h2 * 64, 0))
        exs = []
        for h2 in range(2):
            ex = g.expool.tile([128, 2, 2, NI], F32R, tag="ex")
            nc.scalar.activation(ex[:], sims[h2][:], AFT.Exp, scale=0.125)
            exs.append(ex)

        pa = g.pav.tile([65, 2, NI], F32, tag="pa")
        for h2 in range(2):
            h = hp * 2 + h2
            for n_mm, (pi, jc) in enumerate([(0, 0), (0, 1), (1, 0), (1, 1)]):
                nc.tensor.matmul(
                    pa[:, h2], v_y[pi * 2 + jc][:, h, :], exs[h2][:, pi, jc],
                    start=(n_mm == 0), stop=(n_mm == 3))
        nc.vector.tensor_copy(un[:, hp * 2:hp * 2 + 2], pa[:])
    return un


def _emit_normalize_wo(g, b, ys, unnorm):
    """For a pair of finished window-groups: recip(s), broadcast, normalize,
    Wo projection + bias, store."""
    nc = g.nc
    s_pair = g.spool.tile([2, H * NI], F32, name=f"sm{b}_{ys[0]}", tag="sm")
    for t, yy in enumerate(ys):
        nc.sync.dma_start(s_pair[t:t + 1, :],
                          unnorm[yy][64:65].rearrange("o h i -> o (h i)"))
    s_rec = g.spool.tile([2, H * NI], F32, name=f"sr{b}_{ys[0]}", tag="sr")
    nc.vector.reciprocal(s_rec[:], s_pair[:])
    s_dram = g.dpool.tile([2, H * NI], F32, name=f"sd{b}_{ys[0]}", tag="sd")
    nc.sync.dma_start(s_dram[:], s_rec[:])

    for t, yy in enumerate(ys):
        un = unnorm[yy]
        bc = g.bcpool.tile([64, H, NI], F32, name=f"bc{b}_{yy}", tag="bc")
        nc.sync.dma_start(bc[:].rearrange("p h i -> p (h i)"),
                          _bcast_ap(s_dram[t], 64, H * NI))

        att = g.atpool.tile([128, 4, NI], F32R, name=f"at{b}_{yy}", tag="at")
        odd = g.atpool.tile([64, 4, NI], F32R, name=f"od{b}_{yy}", tag="od")
        # even heads -> partitions 0..63; odd heads -> temp, DMA to 64..127
        nc.gpsimd.tensor_tensor(att[0:64], un[0:64, 0:H:2], bc[:, 0:H:2],
                                mybir.AluOpType.mult)
        nc.gpsimd.tensor_tensor(odd[:], un[0:64, 1:H:2], bc[:, 1:H:2],
                                mybir.AluOpType.mult)
        nc.sync.dma_start(att[64:128], odd[:])

        for mc in range(2):
            po = g.pproj.tile([128, NI], F32, tag="pp")
            for hp in range(4):
                nc.tensor.matmul(po[:], g.wo_sb[:, hp, mc * 128:(mc + 1) * 128],
                                 att[:, hp], start=(hp == 0), stop=(hp == 3))
            ot = g.outpool.tile([128, NI], F32, name=f"ot{b}_{yy}_{mc}", tag="ot")
            nc.scalar.activation(ot[:], po[:], AFT.Identity,
                                 bias=g.bo_sb[:, mc:mc + 1], scale=1.0)
            dst = g.out_d[b].rearrange("(mc p) r (y c) -> p mc r y c",
                                       p=128, c=WIN)
            nc.sync.dma_start(dst[:, mc, :, yy, :],
                              ot[:].rearrange("p (r c) -> p r c", c=WIN))


def build_program():
    nc = bass.Bass("TRN2", target_bir_lowering=False, debug=False, num_devices=N_CORES)
    g = Ctx()
    g.nc = nc

    x_d = nc.dram_tensor("x", [B, C, RS, WCOL], F32R, kind="ExternalInput").ap()
    s_d = nc.dram_tensor("skip", [B, C, RS, WCOL], F32R, kind="ExternalInput").ap()
    wq_d = nc.dram_tensor("wqT", [C, INNER], F32R, kind="ExternalInput").ap()
    wk_d = nc.dram_tensor("wkT", [C, INNER], F32R, kind="ExternalInput").ap()
    wv_d = nc.dram_tensor("wvT", [C, INNER], F32R, kind="ExternalInput").ap()
    wo_d = nc.dram_tensor("woT", [INNER, C], F32R, kind="ExternalInput").ap()
    bo_d = nc.dram_tensor("bo", [C], F32, kind="ExternalInput").ap()
    g.out_d = nc.dram_tensor("out", [B, C, RS, WCOL], F32, kind="ExternalOutput").ap()

    with tile.TileContext(nc) as tc:
        with (
            tc.tile_pool(name="wpool", bufs=1) as wpool,
            tc.tile_pool(name="xpool", bufs=1) as xpool,
            tc.tile_pool(name="xw", bufs=2) as xwpool,
            tc.tile_pool(name="qk", bufs=2) as qkpool,
            tc.tile_pool(name="vp", bufs=6) as vpool,
            tc.tile_pool(name="ex", bufs=2) as expool,
            tc.tile_pool(name="un", bufs=5) as unpool,
            tc.tile_pool(name="at", bufs=2) as atpool,
            tc.tile_pool(name="bc", bufs=1) as bcpool,
            tc.tile_pool(name="sp", bufs=1) as spool,
            tc.tile_pool(name="ou", bufs=2) as outpool,
            tc.tile_pool(name="pproj", bufs=2, space="PSUM") as pproj,
            tc.tile_pool(name="psim", bufs=2, space="PSUM") as psim,
            tc.tile_pool(name="pav", bufs=2, space="PSUM") as pav,
            tc.tile_pool(name="dram", bufs=3, space="DRAM") as dpool,
        ):
            g.xwpool, g.qkpool, g.vpool, g.expool = xwpool, qkpool, vpool, expool
            g.unpool, g.atpool, g.bcpool, g.spool = unpool, atpool, bcpool, spool
            g.outpool, g.pproj, g.psim, g.pav, g.dpool = outpool, pproj, psim, pav, dpool

            g.wq_sb = wpool.tile([128, 2, INNER], F32R, name="wq")
            nc.sync.dma_start(g.wq_sb[:], wq_d.rearrange("(kc p) m -> p kc m", p=128))
            g.wk_sb = wpool.tile([128, 2, INNER], F32R, name="wk")
            nc.sync.dma_start(g.wk_sb[:], wk_d.rearrange("(kc p) m -> p kc m", p=128))
            g.wv_sb = wpool.tile([128, 2, INNER], F32R, name="wv")
            nc.sync.dma_start(g.wv_sb[:], wv_d.rearrange("(kc p) m -> p kc m", p=128))
            g.wo_sb = wpool.tile([128, 4, C], F32R, name="wo")
            nc.sync.dma_start(g.wo_sb[:], wo_d.rearrange("(kc p) m -> p kc m", p=128))
            g.bo_sb = wpool.tile([128, 2], F32, name="bo")
            nc.sync.dma_start(g.bo_sb[:], bo_d.rearrange("(mc p) -> p mc", p=128))

            for b in range(B):
                x_sb = xpool.tile([128, 2, PX], F32R, name=f"x{b}", tag="xs")
                nc.sync.dma_start(x_sb[:],
                                  x_d[b].rearrange("(kc p) r w -> p kc (r w)", p=128))
                sk_sb = xpool.tile([128, 2, PX], F32R, name=f"s{b}", tag="ss")
                nc.sync.dma_start(sk_sb[:],
                                  s_d[b].rearrange("(kc p) r w -> p kc (r w)", p=128))
                g.parts = (x_sb, sk_sb)

                unnorm = {}
                pending = []
                for y in range(NY):
                    q_y, k_y, v_y = _emit_projections(g, b, y)
                    unnorm[y] = _emit_attention(g, b, y, q_y, k_y, v_y)
                    if y % 2 == 1:
                        # pipeline: emit the PREVIOUS pair's normalize now, so
                        # its DMA/recip/GPSIMD chain hides under this pair's
                        # attention and PE never stalls at the Wo matmuls
                        pending.append((y - 1, y))
                        if len(pending) > 1:
                            _emit_normalize_wo(g, b, pending.pop(0), unnorm)
                for ys in pending:
                    _emit_normalize_wo(g, b, ys, unnorm)

    _split_multiwaits(nc)
    return nc


_PROGRAM = None


def _get_program():
    global _PROGRAM
    if _PROGRAM is None:
        _PROGRAM = build_program()
    return _PROGRAM


def kernel(x, skip, Wq, Wkv, Wo, bo):
    x = np.asarray(x, dtype=np.float32)
    skip = np.asarray(skip, dtype=np.float32)
    wqT = np.ascontiguousarray(np.asarray(Wq, np.float32).T)           # [C, INNER]
    wkT = np.ascontiguousarray(np.asarray(Wkv, np.float32)[:INNER].T)  # [C, INNER]
    wvT = np.ascontiguousarray(np.asarray(Wkv, np.float32)[INNER:].T)  # [C, INNER]
    woT = np.ascontiguousarray(np.asarray(Wo, np.float32).T)           # [INNER, C]
    bo = np.ascontiguousarray(np.asarray(bo, np.float32))

    nc = _get_program()
    in_maps = []
    for c in range(N_CORES):
        r0, r1 = c * RS, (c + 1) * RS
        in_maps.append({
            "x": np.ascontiguousarray(x[:, :, r0:r1, :]),
            "skip": np.ascontiguousarray(skip[:, :, r0:r1, :]),
            "wqT": wqT, "wkT": wkT, "wvT": wvT, "woT": woT, "bo": bo,
        })
    res = run_bass_kernel_spmd(nc, in_maps, list(range(N_CORES)))
    out = np.empty((B, C, N_CORES * RS, WCOL), dtype=np.float32)
    for c in range(N_CORES):
        out[:, :, c * RS:(c + 1) * RS, :] = res.results[c]["out"]
    return out


# revision 15
# speedup vs baseline: 5797.7244x; 5797.7244x over previous
"""Windowed sparse attention (16x16 windows, keys from x+skip) on 8 TRN2 NeuronCores.

Reference computation (all 1x1 convs + per-window attention):
  q = Wq @ x;  k,v = split(Wkv @ [x;skip]);  per 16x16 window w/ 256 queries and
  512 keys (256 from x, 256 from skip):  out = softmax(q k^T / 8) v;  y = Wo @ out + bo.

Sharding: each core takes one 16-row strip of the 128x128 image (one window-row X),
both batch elements — all 128 of its windows are fully local; only weights replicated.

Per-core dataflow (fp32r matmuls at full PE rate, transposed-softmax layout):
  - projections produce qT/kT [d, pixels] and v in [pixel, (h d)] layout directly
  - simT[j,i] = kT^T @ qT per window; exp on ScalarE (scale=1/8 folded in)
  - attn@v via lhsT = [v | ones]: softmax denominator s arrives free as psum row 64
  - recip(s) on DVE per window-pair, broadcast via DRAM-bounce DMA,
    normalize-mult on GPSIMD
  - Wo projection from head-stacked normalized outputs, bias fused in ScalarE evac
"""
import sys

if '/opt/trn_rl_repo' not in sys.path:
    sys.path.insert(0, '/opt/trn_rl_repo')

import numpy as np
import concourse.bass as bass
import concourse.tile as tile
import concourse.mybir as mybir
from concourse.bass_utils import run_bass_kernel_spmd

F32 = mybir.dt.float32
F32R = mybir.dt.float32r
AFT = mybir.ActivationFunctionType

N_CORES = 8
B = 2            # batch
C = 256          # model channels
H = 8            # heads
D = 64           # head dim
INNER = H * D    # 512
WIN = 16         # window side
RS = 16          # strip rows per core (= one window row)
WCOL = 128       # image width
PX = RS * WCOL   # 2048 pixels per (batch, strip)
NY = 8           # windows along width
NI = WIN * WIN   # 256 queries per window


def _split_multiwaits(nc, max_waits=1):
    """walrus codegen rejects instructions carrying >1 sem wait (seen on the
    TileContext exit drain); hoist extras onto single-wait NoOps just before."""
    for f in nc.m.functions:
        for blk in f.blocks:
            out, changed = [], False
            for ins in blk.instructions:
                si = ins.sync_info
                if si is not None and len(si.on_wait) > max_waits:
                    waits = list(si.on_wait)
                    SyncInfo = type(si)
                    extra, keep = waits[:-max_waits], waits[-max_waits:]
                    for i, w in enumerate(extra):
                        n = mybir.InstNoOp(name=f"{ins.name}-sw{i}", ins=[], outs=[])
                        n.engine = ins.engine
                        n.sync_info = SyncInfo(on_wait=[w], on_update=[])
                        out.append(n)
                    si.on_wait = keep
                    changed = True
                out.append(ins)
            if changed:
                blk.instructions = out
    return nc


def _bcast_ap(dram_row_ap, parts, free):
    """[free] DRAM row -> [parts, free] replicated read AP (step-0 leading dim)."""
    APcls = type(dram_row_ap)
    return APcls(tensor=dram_row_ap.tensor, offset=dram_row_ap.offset,
                 ap=[[0, parts], [1, free]])


class Ctx:
    pass


def _emit_projections(g, b, y):
    """Stage window pixels and project q/k/v for window-group (b, y)."""
    nc = g.nc
    xw = g.xwpool.tile([128, 2, 2, NI], F32R, name=f"xw{b}_{y}", tag="xw")
    for pi in range(2):
        nc.sync.dma_start(
            xw[:, pi],
            g.parts[pi].rearrange("p kc (r y c) -> p kc y r c",
                                  y=NY, c=WIN)[:, :, y])

    # q: qT [128(2h), NI] x 4 head-pair chunks
    q_y = g.qkpool.tile([128, 4, NI], F32R, name=f"q{b}_{y}", tag="q")
    for mc2 in range(2):
        pq = g.pproj.tile([128, 2, NI], F32, tag="pp")
        for mi in range(2):
            mc = mc2 * 2 + mi
            for kc in range(2):
                nc.tensor.matmul(
                    pq[:, mi], g.wq_sb[:, kc, mc * 128:(mc + 1) * 128],
                    xw[:, 0, kc, :], start=(kc == 0), stop=(kc == 1))
        nc.vector.tensor_copy(q_y[:, mc2 * 2:(mc2 + 1) * 2], pq[:])

    # k: kT [128(2h), 2part, NI] x 4 chunks
    k_y = g.qkpool.tile([128, 4, 2, NI], F32R, name=f"k{b}_{y}", tag="k")
    for kc4 in range(4):
        pk = g.pproj.tile([128, 2, NI], F32, tag="pp")
        for pi in range(2):
            for kc in range(2):
                nc.tensor.matmul(
                    pk[:, pi], g.wk_sb[:, kc, kc4 * 128:(kc4 + 1) * 128],
                    xw[:, pi, kc, :], start=(kc == 0), stop=(kc == 1))
        nc.vector.tensor_copy(k_y[:, kc4], pk[:])

    # v (transposed): [128 j, 8h x (64|ones)] x 4 j-chunks
    v_y = []
    for pi in range(2):
        for jc in range(2):
            pv = g.pproj.tile([128, INNER], F32, tag="pp")
            for kc in range(2):
                nc.tensor.matmul(
                    pv[:], xw[:, pi, kc, jc * 128:(jc + 1) * 128],
                    g.wv_sb[:, kc, :], start=(kc == 0), stop=(kc == 1))
            vt = g.vpool.tile([128, H, D + 1], F32R,
                              name=f"v{b}_{y}_{pi}{jc}", tag="v")
            nc.vector.tensor_copy(vt[:, :, D], nc.const_aps.tensor(1.0, (128, H), F32))
            nc.vector.tensor_copy(
                vt[:, :, 0:D], pv[:].rearrange("p (h d) -> p h d", h=H))
            v_y.append(vt)
    return q_y, k_y, v_y


def _emit_attention(g, b, y, q_y, k_y, v_y):
    """simT -> exp -> (attn@[v|1]) for all 8 heads; returns unnorm [65, H, NI]."""
    nc = g.nc
    un = g.unpool.tile([65, H, NI], F32, name=f"u{b}_{y}", tag="un")
    for hp in range(4):
        # per-head sim tiles (2 banks each) so exp(h) overlaps sim MMs(h+1);
        # even/odd heads packed in the PE array via tile_position row groups
        sims = [g.psim.tile([128, 2, 2, NI], F32, tag="sim", name=f"sim{b}_{y}_{hp}_{h2}")
                for h2 in range(2)]
        for pi in range(2):
            for jh in range(2):
                for h2 in range(2):
                    nc.tensor.matmul(
                        sims[h2][:, pi, jh],
                        k_y[h2 * 64:(h2 + 1) * 64, hp, pi, jh * 128:(jh + 1) * 128],
                        q_y[h2 * 64:(h2 + 1) * 64, hp],
                        start=True, stop=True, tile_position=(h2 * 64, 0))
        exs = []
        for h2 in range(2):
            ex = g.expool.tile([128, 2, 2, NI], F32R, tag="ex")
            nc.scalar.activation(ex[:], sims[h2][:], AFT.Exp, scale=0.125)
            exs.append(ex)

        pa = g.pav.tile([65, 2, NI], F32, tag="pa")
        for h2 in range(2):
            h = hp * 2 + h2
            for n_mm, (pi, jc) in enumerate([(0, 0), (0, 1), (1, 0), (1, 1)]):
                nc.tensor.matmul(
                    pa[:, h2], v_y[pi * 2 + jc][:, h, :], exs[h2][:, pi, jc],
                    start=(n_mm == 0), stop=(n_mm == 3))
        nc.vector.tensor_copy(un[:, hp * 2:hp * 2 + 2], pa[:])
    return un


def _emit_normalize_wo(g, b, ys, unnorm):
    """For a pair of finished window-groups: recip(s), broadcast, normalize,
    Wo projection + bias, store."""
    nc = g.nc
    s_pair = g.spool.tile([2, H * NI], F32, name=f"sm{b}_{ys[0]}", tag="sm")
    for t, yy in enumerate(ys):
        nc.sync.dma_start(s_pair[t:t + 1, :],
                          unnorm[yy][64:65].rearrange("o h i -> o (h i)"))
    s_rec = g.spool.tile([2, H * NI], F32, name=f"sr{b}_{ys[0]}", tag="sr")
    nc.vector.reciprocal(s_rec[:], s_pair[:])
    s_dram = g.dpool.tile([2, H * NI], F32, name=f"sd{b}_{ys[0]}", tag="sd")
    nc.sync.dma_start(s_dram[:], s_rec[:])

    for t, yy in enumerate(ys):
        un = unnorm[yy]
        bc = g.bcpool.tile([64, H, NI], F32, name=f"bc{b}_{yy}", tag="bc")
        nc.sync.dma_start(bc[:].rearrange("p h i -> p (h i)"),
                          _bcast_ap(s_dram[t], 64, H * NI))

        att = g.atpool.tile([128, 4, NI], F32R, name=f"at{b}_{yy}", tag="at")
        odd = g.atpool.tile([64, 4, NI], F32R, name=f"od{b}_{yy}", tag="od")
        # even heads -> partitions 0..63; odd heads -> temp, DMA to 64..127
        nc.gpsimd.tensor_tensor(att[0:64], un[0:64, 0:H:2], bc[:, 0:H:2],
                                mybir.AluOpType.mult)
        nc.gpsimd.tensor_tensor(odd[:], un[0:64, 1:H:2], bc[:, 1:H:2],
                                mybir.AluOpType.mult)
        nc.sync.dma_start(att[64:128], odd[:])

        for mc in range(2):
            po = g.pproj.tile([128, NI], F32, tag="pp")
            for hp in range(4):
                nc.tensor.matmul(po[:], g.wo_sb[:, hp, mc * 128:(mc + 1) * 128],
                                 att[:, hp], start=(hp == 0), stop=(hp == 3))
            ot = g.outpool.tile([128, NI], F32, name=f"ot{b}_{yy}_{mc}", tag="ot")
            nc.scalar.activation(ot[:], po[:], AFT.Identity,
                                 bias=g.bo_sb[:, mc:mc + 1], scale=1.0)
            dst = g.out_d[b].rearrange("(mc p) r (y c) -> p mc r y c",
                                       p=128, c=WIN)
            nc.sync.dma_start(dst[:, mc, :, yy, :],
                              ot[:].rearrange("p (r c) -> p r c", c=WIN))


def build_program(reps=1):
    nc = bass.Bass("TRN2", target_bir_lowering=False, debug=False, num_devices=N_CORES)
    g = Ctx()
    g.nc = nc

    x_d = nc.dram_tensor("x", [B, C, RS, WCOL], F32R, kind="ExternalInput").ap()
    s_d = nc.dram_tensor("skip", [B, C, RS, WCOL], F32R, kind="ExternalInput").ap()
    wq_d = nc.dram_tensor("wqT", [C, INNER], F32R, kind="ExternalInput").ap()
    wk_d = nc.dram_tensor("wkT", [C, INNER], F32R, kind="ExternalInput").ap()
    wv_d = nc.dram_tensor("wvT", [C, INNER], F32R, kind="ExternalInput").ap()
    wo_d = nc.dram_tensor("woT", [INNER, C], F32R, kind="ExternalInput").ap()
    bo_d = nc.dram_tensor("bo", [C], F32, kind="ExternalInput").ap()
    g.out_d = nc.dram_tensor("out", [B, C, RS, WCOL], F32, kind="ExternalOutput").ap()

    with tile.TileContext(nc) as tc:
        with (
            tc.tile_pool(name="wpool", bufs=1) as wpool,
            tc.tile_pool(name="xpool", bufs=1) as xpool,
            tc.tile_pool(name="xw", bufs=2) as xwpool,
            tc.tile_pool(name="qk", bufs=2) as qkpool,
            tc.tile_pool(name="vp", bufs=6) as vpool,
            tc.tile_pool(name="ex", bufs=2) as expool,
            tc.tile_pool(name="un", bufs=5) as unpool,
            tc.tile_pool(name="at", bufs=2) as atpool,
            tc.tile_pool(name="bc", bufs=1) as bcpool,
            tc.tile_pool(name="sp", bufs=1) as spool,
            tc.tile_pool(name="ou", bufs=2) as outpool,
            tc.tile_pool(name="pproj", bufs=2, space="PSUM") as pproj,
            tc.tile_pool(name="psim", bufs=2, space="PSUM") as psim,
            tc.tile_pool(name="pav", bufs=2, space="PSUM") as pav,
            tc.tile_pool(name="dram", bufs=3, space="DRAM") as dpool,
        ):
            g.xwpool, g.qkpool, g.vpool, g.expool = xwpool, qkpool, vpool, expool
            g.unpool, g.atpool, g.bcpool, g.spool = unpool, atpool, bcpool, spool
            g.outpool, g.pproj, g.psim, g.pav, g.dpool = outpool, pproj, psim, pav, dpool

            g.wq_sb = wpool.tile([128, 2, INNER], F32R, name="wq")
            nc.sync.dma_start(g.wq_sb[:], wq_d.rearrange("(kc p) m -> p kc m", p=128))
            g.wk_sb = wpool.tile([128, 2, INNER], F32R, name="wk")
            nc.sync.dma_start(g.wk_sb[:], wk_d.rearrange("(kc p) m -> p kc m", p=128))
            g.wv_sb = wpool.tile([128, 2, INNER], F32R, name="wv")
            nc.sync.dma_start(g.wv_sb[:], wv_d.rearrange("(kc p) m -> p kc m", p=128))
            g.wo_sb = wpool.tile([128, 4, C], F32R, name="wo")
            nc.sync.dma_start(g.wo_sb[:], wo_d.rearrange("(kc p) m -> p kc m", p=128))
            g.bo_sb = wpool.tile([128, 2], F32, name="bo")
            nc.sync.dma_start(g.bo_sb[:], bo_d.rearrange("(mc p) -> p mc", p=128))

            def _body():
                for b in range(B):
                    _emit_batch(g, b, x_d, s_d, xpool)

            if reps == 1:
                _body()
            else:
                with tc.For_i(0, reps, 1):
                    _body()

    _split_multiwaits(nc)
    return nc


def _emit_batch(g, b, x_d, s_d, xpool):
    nc = g.nc
    if True:
            if True:
                x_sb = xpool.tile([128, 2, PX], F32R, name=f"x{b}", tag="xs")
                nc.sync.dma_start(x_sb[:],
                                  x_d[b].rearrange("(kc p) r w -> p kc (r w)", p=128))
                sk_sb = xpool.tile([128, 2, PX], F32R, name=f"s{b}", tag="ss")
                nc.sync.dma_start(sk_sb[:],
                                  s_d[b].rearrange("(kc p) r w -> p kc (r w)", p=128))
                g.parts = (x_sb, sk_sb)

                unnorm = {}
                pending = []
                for y in range(NY):
                    q_y, k_y, v_y = _emit_projections(g, b, y)
                    unnorm[y] = _emit_attention(g, b, y, q_y, k_y, v_y)
                    if y % 2 == 1:
                        # pipeline: emit the PREVIOUS pair's normalize now, so
                        # its DMA/recip/GPSIMD chain hides under this pair's
                        # attention and PE never stalls at the Wo matmuls
                        pending.append((y - 1, y))
                        if len(pending) > 1:
                            _emit_normalize_wo(g, b, pending.pop(0), unnorm)
                for ys in pending:
                    _emit_normalize_wo(g, b, ys, unnorm)


_PROGRAM = None


def _get_program():
    global _PROGRAM
    if _PROGRAM is None:
        _PROGRAM = build_program()
    return _PROGRAM


def kernel(x, skip, Wq, Wkv, Wo, bo):
    x = np.asarray(x, dtype=np.float32)
    skip = np.asarray(skip, dtype=np.float32)
    wqT = np.ascontiguousarray(np.asarray(Wq, np.float32).T)           # [C, INNER]
    wkT = np.ascontiguousarray(np.asarray(Wkv, np.float32)[:INNER].T)  # [C, INNER]
    wvT = np.ascontiguousarray(np.asarray(Wkv, np.float32)[INNER:].T)  # [C, INNER]
    woT = np.ascontiguousarray(np.asarray(Wo, np.float32).T)           # [INNER, C]
    bo = np.ascontiguousarray(np.asarray(bo, np.float32))

    nc = _get_program()
    in_maps = []
    for c in range(N_CORES):
        r0, r1 = c * RS, (c + 1) * RS
        in_maps.append({
            "x": np.ascontiguousarray(x[:, :, r0:r1, :]),
            "skip": np.ascontiguousarray(skip[:, :, r0:r1, :]),
            "wqT": wqT, "wkT": wkT, "wvT": wvT, "woT": woT, "bo": bo,
        })
    res = run_bass_kernel_spmd(nc, in_maps, list(range(N_CORES)))
    out = np.empty((B, C, N_CORES * RS, WCOL), dtype=np.float32)
    for c in range(N_CORES):
        out[:, :, c * RS:(c + 1) * RS, :] = res.results[c]["out"]
    return out
